# revision 1
# baseline (speedup 1.0000x reference)
"""Trainium2 Bass kernel for Bottleneck+DynamicConv (B=16,C=256,H=W=64,E=4).

Data-parallel over batch: 8 NeuronCores x 2 samples each. Each 3x3 conv is
128x128x512 matmuls: weights stationary per (tap, cin-tile, cout-tile)
block, the moving operand is a 3D access pattern [128, 8 rows, 64 cols]
into a zero-padded 66x66 image held in SBUF, so taps are pure AP offsets
and only valid output pixels are computed. BN scales are folded into conv
weights on the host; the BN bias + SiLU epilogue runs fused on the scalar
engine during PSUM evacuation. Compute dtype fp16 (same PE rate as bf16,
8x finer rounding), accumulation fp32 in PSUM.

On top of that baseline (273us):
- Mixed-precision taps via fp8e4 DoubleRow matmuls (contraction 256, 2x
  MAC rate, 4D moving AP [128, 2, 8, 64]) accumulated into the same fp32
  psum as the fp16 matmuls, natural-scale e4m3 operands: the center tap
  in every psum group of both convs, plus the top-center tap on 5 of 8
  of conv1's row-blocks. Groups drop from 18 to 16 (14) fp16 + 1 (2) DR
  slots (~-15us). Measured rel err 1.868e-2 vs the 2e-2 gate; hardware
  matches the numpy emulation (fp8_exp.py) to 5 digits across runs.
- HAM warmup: 36 dummy matmuls issued first burn the NEFF-preamble DMA
  wait and flip the PE clock-gate to 8/8 (2.4GHz) before real work.
- Startup DMA in consumption order across both HWDGE rings, first psum
  group's operands (w1 block 0 + image rows 0-9) leading.
- fp16 output path (epilogue, residual add, writeback) on 3 DMA queues;
  the last tile drains in quarter chunks with a split final activation.
Result: 254.6-257.5us across runs (startup DMA/HAM phase variance);
tensor engine fully dense between first and last matmul.
"""

from contextlib import ExitStack

import ml_dtypes
import numpy as np

import concourse.bacc as bacc
import concourse.bass as bass
import concourse.mybir as mybir
from concourse import tile
from concourse.bass_utils import run_bass_kernel_spmd

B, C, H, W, E = 16, 256, 64, 64, 4
KH = KW = 3
EPS = 1e-5
NCORES = 8
S = B // NCORES           # samples per core = 2
CT = C // 128             # channel tiles = 2
PD = W + 2                # padded width/height = 66
PF = PD * PD              # padded flat pixels per channel tile = 4356
NGB = 8                   # row-blocks per (sample, cout tile)
RB = H // NGB             # rows per block = 8
NN = RB * W               # matmul free dim = 512
HWF = H * W               # 4096
NBLK = CT * 9 * CT        # 36 weight blocks of [128,128]
BLKF = NBLK * 128         # 4608 weight columns
F16 = mybir.dt.float16
F32 = mybir.dt.float32
F8 = mybir.dt.float8e4
NPF16 = np.float16
NPF8 = ml_dtypes.float8_e4m3fn
# the center 3x3 tap: its two cin-tile blocks are computed as ONE fp8
# DoubleRow matmul (2 planes, contraction 256) at natural scale, accumulated
# into the same fp32 psum as the 16 fp16 matmuls. Saves ~5% PE time for a
# measured rel-err of 1.64e-2 (gate 2e-2). The center tap never reads the
# pad ring, so the fp8 copies need no pad maintenance.
PAIR_T = 4
# second fp8 pair (top-center tap) on conv1's even row-blocks only: total
# measured rel err 1.823e-2, still under the 2e-2 gate
PAIR_T2 = 1
DR = mybir.MatmulPerfMode.DoubleRow

TRACE = False
LAST_EXEC_NS = None
# swappable for simulator testing (CoreSim has no Silu); HW uses native Silu
ACT_FUNC = mybir.ActivationFunctionType.Silu

_prog_cache = {}


def _blk(o, t, ci):
    # column block index in the packed weight layout [128, 36*128]
    return (o * 9 + t) * 2 + ci


def _emit_conv(nc, ps_pool, wcol, views, epilogue, w8col, v8, w8col2=None):
    """One conv layer for one sample. wcol(o,t,ci) -> lhsT AP;
    views[ci] = [128, 66, 66] padded-input AP; epilogue(o, gb, psum_ap);
    w8col(o) -> [128, 2, 128] fp8 pair lhsT; v8 = [128, 2, 66, 66] fp8.
    w8col2: optional second fp8 pair (tap PAIR_T2) applied on even gb."""
    for o in range(CT):
        for gb in range(NGB):
            second = w8col2 is not None and (gb % 2 == 0 or gb == 1)
            r0 = gb * RB
            ps = ps_pool.tile([128, NN], F32, tag="ps")
            idx = 0
            for ci in range(CT):
                for t in range(9):
                    if t == PAIR_T or (second and t == PAIR_T2):
                        continue
                    dy, dx = t // 3 - 1, t % 3 - 1
                    rhs = views[ci][:, r0 + 1 + dy:r0 + 1 + dy + RB,
                                    1 + dx:1 + dx + W]
                    nc.tensor.matmul(
                        ps[:], wcol(o, t, ci), rhs,
                        start=(idx == 0), stop=False)
                    idx += 1
            if second:
                # PAIR_T2 = 1 is (dy=-1, dx=0): padded rows r0..r0+7
                nc.tensor.matmul(
                    ps[:], w8col2(o), v8[:, :, r0:r0 + RB, 1:1 + W],
                    start=False, stop=False, perf_mode=DR)
            nc.tensor.matmul(
                ps[:], w8col(o), v8[:, :, r0 + 1:r0 + 1 + RB, 1:1 + W],
                start=False, stop=True, perf_mode=DR)
            epilogue(o, gb, ps)


def _build_program():
    nc = bacc.Bacc(
        "TRN2", target_bir_lowering=False, debug=False,
        enable_asserts=False, num_devices=NCORES)

    xpad_d = nc.dram_tensor("xpad", [S, CT, 128, PF], F16, kind="ExternalInput")
    x8_d = nc.dram_tensor("x8", [S, CT, 128, PF], F8, kind="ExternalInput")
    w1p8_d = nc.dram_tensor(
        "w1p8", [128, 2 * CT * 256], F8, kind="ExternalInput")
    w1_d = nc.dram_tensor("w1sb", [128, BLKF], F16, kind="ExternalInput")
    bank_d = nc.dram_tensor("bank", [128, E * BLKF], F16, kind="ExternalInput")
    wr_d = nc.dram_tensor("wrt", [128, CT * E], F32, kind="ExternalInput")
    br_d = nc.dram_tensor("brb", [128, E], F32, kind="ExternalInput")
    b1_d = nc.dram_tensor("b1sb", [128, CT], F32, kind="ExternalInput")
    b2_d = nc.dram_tensor("b2sb", [128, CT], F32, kind="ExternalInput")
    out_d = nc.dram_tensor("out", [S, CT, 128, HWF], F16, kind="ExternalOutput")

    with tile.TileContext(nc) as tc, ExitStack() as ctx:
        const = ctx.enter_context(tc.tile_pool(name="const", bufs=1))
        xp_pool = ctx.enter_context(tc.tile_pool(name="xp", bufs=2))
        x8_pool = xp_pool
        yp_pool = ctx.enter_context(tc.tile_pool(name="yp", bufs=2))
        y8_pool = yp_pool
        kern_pool = ctx.enter_context(tc.tile_pool(name="kern", bufs=2))
        outp_pool = ctx.enter_context(tc.tile_pool(name="outp", bufs=2))
        small = ctx.enter_context(tc.tile_pool(name="small", bufs=4))
        ps_pool = ctx.enter_context(tc.tile_pool(name="ps", bufs=4, space="PSUM"))
        psr_pool = ctx.enter_context(tc.tile_pool(name="psr", bufs=2, space="PSUM"))
        warm_pool = psr_pool

        # HAM warmup: the PE clock-gate defaults to 4/8 (1.2 GHz) and only
        # releases after ~3.4us of sustained busy. Real matmuls can't start
        # until ~9us (NEFF preamble + first DMA), so burn the DMA-wait window
        # on dummy matmuls; the real stream then runs warm (2.4 GHz) from
        # its first instruction.
        dummy_t = const.tile([128, 128], F16)
        nc.gpsimd.memset(dummy_t[:], 0.0)
        warm_ps = warm_pool.tile([128, 128], F32, tag="warm")
        for _ in range(36):
            nc.tensor.matmul(
                warm_ps[:], dummy_t[:], dummy_t[:], start=True, stop=True)

        # startup-critical DMA order: the very first matmuls need only the
        # o=0 weight blocks and the leading image rows of sample 0 — load
        # those first (w1 first half split across both rings) so the PE
        # starts as soon as the preamble allows
        w1_t = const.tile([128, BLKF], F16)
        H1 = BLKF // 4
        nc.scalar.dma_start(w1_t[:, 0:640], w1_d.ap()[:, 0:640])
        b1_t = const.tile([128, CT], F32)
        nc.scalar.dma_start(b1_t[:], b1_d.ap())
        w1p8_t = const.tile([128, 2 * CT * 256], F8)
        nc.scalar.dma_start(w1p8_t[:], w1p8_d.ap())
        wr_t = const.tile([128, CT * E], F32)
        br_t = const.tile([128, E], F32)
        b2_t = const.tile([128, CT], F32)
        ones_t = const.tile([128, 128], F32)
        nc.vector.memset(ones_t[:], 1.0)
        # the big expert bank is only needed after conv1(s0): allocate now,
        # DMA later so it doesn't delay the startup-critical loads above
        bank_t = const.tile([128, E * BLKF], F16)

        # s0 image quarters land in consumption order (row-blocks 0..1 need
        # rows <18, blocks 2..3 rows <34, ...)
        QS = [0, 18 * PD, 34 * PD, 50 * PD, PF]
        XH = 34 * PD  # x8 first-half split point (rows 0..33)
        xpts, ypts, y8ts, kerns, kern8s = [], [], [], [], []
        for s in range(S):
            xpt = xp_pool.tile([128, CT * PF], F16, tag="xp")
            xpts.append(xpt)
            x8t = x8_pool.tile([128, CT * PF], F8, tag="x8")
            if s == 0:
                # explicit per-ring schedules, in consumption order. sync
                # leads with image q0 (first matmul's gate); fp8 pieces are
                # placed just ahead of the DoubleRow matmuls that read them.
                XQ = 18 * PD
                for eng, job in (
                    (nc.sync, [
                        (xpt, 0, 0, 10 * PD, xpad_d),     # rows 0-9: gates
                        (w1_t, None, H1, H1 + 640, w1_d),  # the whole first
                        (xpt, 0, 10 * PD, XQ, xpad_d),    # psum group
                        (x8t, 1, 0, XQ, x8_d),            # fp8 head ci1
                        (w1_t, None, H1 + 640, 2 * H1, w1_d),
                        (xpt, 0, XQ, XH, xpad_d),         # image q1 ci0
                        (xpt, 0, XH, 50 * PD, xpad_d),    # image q2 ci0
                        (w1_t, None, 2 * H1, 3 * H1, w1_d),
                        (x8t, 1, XQ, 50 * PD, x8_d),      # fp8 mid ci1
                        (xpt, 0, 50 * PD, PF, xpad_d),    # image q3 ci0
                        (x8t, 1, 50 * PD, PF, x8_d),      # fp8 tail ci1
                    ]),
                    (nc.scalar, [
                        (xpt, 1, 0, 10 * PD, xpad_d),     # image q0a ci1
                        (w1_t, None, 640, H1, w1_d),      # h1a second part
                        (x8t, 0, 0, XQ, x8_d),            # fp8 head ci0
                        (xpt, 1, 10 * PD, XQ, xpad_d),    # image q0b ci1
                        (xpt, 1, XQ, XH, xpad_d),         # image q1 ci1
                        (x8t, 0, XQ, 50 * PD, x8_d),      # fp8 mid ci0
                        (xpt, 1, XH, 50 * PD, xpad_d),    # image q2 ci1
                        (w1_t, None, 3 * H1, 4 * H1, w1_d),
                        (xpt, 1, 50 * PD, PF, xpad_d),    # image q3 ci1
                        (x8t, 0, 50 * PD, PF, x8_d),      # fp8 tail ci0
                    ]),
                ):
                    for tgt, ci, lo, hi, src in job:
                        if ci is None:
                            eng.dma_start(tgt[:, lo:hi], src.ap()[:, lo:hi])
                        else:
                            eng.dma_start(
                                tgt[:, ci * PF + lo:ci * PF + hi],
                                src.ap()[s, ci][:, lo:hi])
                nc.sync.dma_start(wr_t[:], wr_d.ap())
                nc.sync.dma_start(br_t[:], br_d.ap())
                nc.sync.dma_start(b2_t[:], b2_d.ap())
            else:
                for ci in range(CT):
                    eng = nc.sync if ci == 0 else nc.scalar
                    eng.dma_start(
                        xpt[:, ci * PF:(ci + 1) * PF], xpad_d.ap()[s, ci])
                    eng.dma_start(
                        x8t[:, ci * PF:(ci + 1) * PF], x8_d.ap()[s, ci])
            xviews = [xpt[:, ci * PF:(ci + 1) * PF].rearrange(
                "p (h w) -> p h w", h=PD) for ci in range(CT)]
            x8v = x8t.rearrange("p (c h w) -> p c h w", c=CT, h=PD)

            ypt = yp_pool.tile([128, CT * PF], F16, tag="yp")
            ypts.append(ypt)
            yviews = [ypt[:, ci * PF:(ci + 1) * PF].rearrange(
                "p (h w) -> p h w", h=PD) for ci in range(CT)]
            y8t = y8_pool.tile([128, CT * PF], F8, tag="y8")
            y8ts.append(y8t)
            y8views = [y8t[:, ci * PF:(ci + 1) * PF].rearrange(
                "p (h w) -> p h w", h=PD) for ci in range(CT)]

            def epi1(o, gb, ps, yviews=yviews, y8views=y8views):
                r0 = gb * RB
                psv = ps[:].rearrange("p (a b) -> p a b", a=RB)
                nc.scalar.activation(
                    yviews[o][:, r0 + 1:r0 + 1 + RB, 1:1 + W],
                    psv, ACT_FUNC, bias=b1_t[:, o:o + 1])
                # second drain of the same psum: fp8 copy of y for the
                # conv2 DoubleRow pair (natural scale)
                nc.scalar.activation(
                    y8views[o][:, r0 + 1:r0 + 1 + RB, 1:1 + W],
                    psv, ACT_FUNC, bias=b1_t[:, o:o + 1])

            _emit_conv(
                nc, ps_pool,
                lambda o, t, ci: w1_t[:, _blk(o, t, ci) * 128:(_blk(o, t, ci) + 1) * 128],
                xviews, epi1,
                lambda o: w1p8_t[:, o * 256:(o + 1) * 256].rearrange(
                    "p (k m) -> p k m", k=2),
                x8v,
                w8col2=lambda o: w1p8_t[
                    :, CT * 256 + o * 256:CT * 256 + (o + 1) * 256].rearrange(
                    "p (k m) -> p k m", k=2))

            if s == 0:
                # split the 4.7MB bank across both rings; it's only needed
                # once routing(s0) completes (~75us in)
                EB = E * BLKF // 2
                nc.sync.dma_start(bank_t[:, 0:EB], bank_d.ap()[:, 0:EB])
                nc.scalar.dma_start(bank_t[:, EB:], bank_d.ap()[:, EB:])

            # zero the pad ring of y so conv2 sees proper zero padding
            for ci in range(CT):
                yv = yviews[ci]
                nc.vector.memset(yv[:, 0:1, :], 0.0)
                nc.vector.memset(yv[:, PD - 1:PD, :], 0.0)
                nc.vector.memset(yv[:, :, 0:1], 0.0)
                nc.vector.memset(yv[:, :, PD - 1:PD], 0.0)

            # routing: pooled mean -> sigmoid(pooled @ wrT + br), all 128
            # partitions carry identical copies (broadcast via ones-matmul)
            psr = psr_pool.tile([128, E], F32, tag="psr")
            for ci in range(CT):
                pooled = small.tile([128, 1], F32, tag="pooled")
                nc.vector.tensor_reduce(
                    pooled[:], yviews[ci][:, 1:1 + H, 1:1 + W],
                    axis=mybir.AxisListType.XY, op=mybir.AluOpType.add)
                pbc = small.tile([128, 128], F32, tag="pbc")
                nc.vector.tensor_scalar_mul(pbc[:], ones_t[:], pooled[:, 0:1])
                nc.tensor.matmul(
                    psr[:], pbc[:], wr_t[:, ci * E:(ci + 1) * E],
                    start=(ci == 0), stop=(ci == CT - 1))
            logits = small.tile([128, E], F32, tag="logits")
            nc.vector.tensor_add(logits[:], psr[:], br_t[:])
            r_t = small.tile([128, E], F32, tag="r")
            nc.scalar.activation(
                r_t[:], logits[:], mybir.ActivationFunctionType.Sigmoid)

            # expert-weighted kernel bank mix: kern = sum_e r_e * bank_e
            kt = kern_pool.tile([128, BLKF], F16, tag="kern")
            kerns.append(kt)
            nc.vector.tensor_scalar_mul(kt[:], bank_t[:, 0:BLKF], r_t[:, 0:1])
            for e in range(1, E):
                nc.vector.scalar_tensor_tensor(
                    kt[:], bank_t[:, e * BLKF:(e + 1) * BLKF], r_t[:, e:e + 1],
                    kt[:], mybir.AluOpType.mult, mybir.AluOpType.add)
            # fp8 mix of the center-tap block pairs (final stt casts to fp8)
            kt8 = kern_pool.tile([128, CT * 256], F8, tag="kern8")
            kern8s.append(kt8)
            for o in range(CT):
                base = _blk(o, PAIR_T, 0) * 128
                off = [e * BLKF + base for e in range(E)]
                tmp = small.tile([128, 256], F16, tag="kt8tmp")
                nc.vector.tensor_scalar_mul(
                    tmp[:], bank_t[:, off[0]:off[0] + 256], r_t[:, 0:1])
                for e in range(1, E - 1):
                    nc.vector.scalar_tensor_tensor(
                        tmp[:], bank_t[:, off[e]:off[e] + 256], r_t[:, e:e + 1],
                        tmp[:], mybir.AluOpType.mult, mybir.AluOpType.add)
                nc.vector.scalar_tensor_tensor(
                    kt8[:, o * 256:(o + 1) * 256],
                    bank_t[:, off[E - 1]:off[E - 1] + 256], r_t[:, E - 1:E],
                    tmp[:], mybir.AluOpType.mult, mybir.AluOpType.add)

        for s in range(S):
            ypt, kt, kt8 = ypts[s], kerns[s], kern8s[s]
            yviews = [ypt[:, ci * PF:(ci + 1) * PF].rearrange(
                "p (h w) -> p h w", h=PD) for ci in range(CT)]
            y8v = y8ts[s].rearrange("p (c h w) -> p c h w", c=CT, h=PD)
            outps = {}

            def epi2(o, gb, ps):
                if gb == 0:
                    outps[o] = outp_pool.tile(
                        [128, HWF], F16, tag="outp", name=f"outp_s{s}_o{o}")
                if s == S - 1 and o == CT - 1 and gb == NGB - 1:
                    # split the very last drain so the residual-add + DMA
                    # of the first half overlaps the second half
                    for hf in range(2):
                        hs = bass.ts(hf, NN // 2)
                        nc.scalar.activation(
                            outps[o][:, gb * NN + hf * NN // 2:
                                     gb * NN + (hf + 1) * NN // 2],
                            ps[:, hs], ACT_FUNC, bias=b2_t[:, o:o + 1])
                else:
                    nc.scalar.activation(
                        outps[o][:, gb * NN:(gb + 1) * NN], ps[:],
                        ACT_FUNC, bias=b2_t[:, o:o + 1])

            _emit_conv(
                nc, ps_pool,
                lambda o, t, ci: kt[:, _blk(o, t, ci) * 128:(_blk(o, t, ci) + 1) * 128],
                yviews, epi2,
                lambda o: kt8[:, o * 256:(o + 1) * 256].rearrange(
                    "p (k m) -> p k m", k=2),
                y8v)

            # residual add (from the fp16 padded x already in SBUF) +
            # writeback, chunked so the tail overlaps compute; fp16 output
            # halves writeback bytes and doubles DVE add throughput. DMA
            # triggers alternate between the SP ring and the idle GpSimd
            # queue so the tail drains on two queues in parallel.
            xpt = xpts[s]
            xviews2 = [xpt[:, ci * PF:(ci + 1) * PF].rearrange(
                "p (h w) -> p h w", h=PD) for ci in range(CT)]
            for o in range(CT):
                # the very last tile drains in quarter-row-block chunks over
                # three queues so the add->dma->drain tail pipelines
                nch = 16 if (s == S - 1 and o == CT - 1) else 8
                engs = ([nc.gpsimd, nc.sync] * 6 +
                        [nc.scalar, nc.sync, nc.scalar, nc.sync]
                        if nch == 16 else [nc.gpsimd, nc.sync] * 4)
                cw = HWF // nch
                rb = RB * 8 // nch
                for bk in range(nch):
                    sl = bass.ts(bk, cw)
                    nc.vector.tensor_add(
                        outps[o][:, sl].rearrange("p (a b) -> p a b", a=rb),
                        outps[o][:, sl].rearrange("p (a b) -> p a b", a=rb),
                        xviews2[o][:, 1 + bk * rb:1 + (bk + 1) * rb, 1:1 + W])
                    engs[bk % len(engs)].dma_start(
                        out_d.ap()[s, o][:, sl], outps[o][:, sl])

    nc.compile()
    return nc


def _get_program():
    if "nc" not in _prog_cache:
        _prog_cache["nc"] = _build_program()
    return _prog_cache["nc"]


def kernel(x, w1, bn1_g, bn1_b, bn1_m, bn1_v, wr, br, w_e,
           bn2_g, bn2_b, bn2_m, bn2_v):
    global LAST_EXEC_NS
    f32 = np.float32
    x = np.ascontiguousarray(np.asarray(x, f32))
    w1 = np.asarray(w1, f32)
    wr = np.asarray(wr, f32)
    br = np.asarray(br, f32)
    w_e = np.asarray(w_e, f32)

    s1 = np.asarray(bn1_g, f32) / np.sqrt(np.asarray(bn1_v, f32) + EPS)
    b1 = np.asarray(bn1_b, f32) - np.asarray(bn1_m, f32) * s1
    s2 = np.asarray(bn2_g, f32) / np.sqrt(np.asarray(bn2_v, f32) + EPS)
    b2 = np.asarray(bn2_b, f32) - np.asarray(bn2_m, f32) * s2

    # pack conv1 weights [cout, cin, ky, kx] (BN1 scale folded) into the
    # lhsT block layout: [cin128 partitions, (o, ky, kx, ci, cout128)]
    w1f = w1 * s1[:, None, None, None]
    w1sb = np.ascontiguousarray(
        w1f.reshape(CT, 128, CT, 128, KH, KW)
        .transpose(3, 0, 4, 5, 2, 1).reshape(128, BLKF)).astype(NPF16)

    # expert bank likewise (BN2 scale folded), one block set per expert
    wef = w_e.reshape(E, C, C, KH, KW) * s2[None, :, None, None, None]
    bank = np.ascontiguousarray(
        wef.reshape(E, CT, 128, CT, 128, KH, KW)
        .transpose(4, 0, 1, 5, 6, 3, 2).reshape(128, E * BLKF)).astype(NPF16)

    # routing weights with the 1/(H*W) mean folded in: [p, (ci, e)]
    wrt = np.ascontiguousarray(
        (wr / HWF).reshape(E, CT, 128).transpose(2, 1, 0).reshape(128, CT * E))
    brb = np.ascontiguousarray(np.broadcast_to(br, (128, E)))
    b1sb = np.ascontiguousarray(b1.reshape(CT, 128).T)
    b2sb = np.ascontiguousarray(b2.reshape(CT, 128).T)

    # padded fp16 x for the conv matmuls (also reused as the residual)
    pad = np.zeros((B, CT, 128, PD, PD), f32)
    pad[:, :, :, 1:H + 1, 1:W + 1] = x.reshape(B, CT, 128, H, W)
    padf = pad.reshape(B, CT, 128, PF)
    xpad = np.ascontiguousarray(padf.astype(NPF16))
    # natural-scale fp8 copy for the center-tap DoubleRow pair (TRN e4m3
    # matches OCP e4m3fn within +-240)
    x8 = np.ascontiguousarray(np.clip(padf, -240, 240).astype(NPF8))

    # fp8 weight block pairs for taps PAIR_T (center) and PAIR_T2
    # (top-center), lhsT layout [cin_p, (tap, o, ci, cout)]
    w1r = w1f.reshape(CT, 128, CT, 128, KH, KW)
    w1p8 = np.ascontiguousarray(np.clip(np.concatenate(
        [w1r[:, :, :, :, t // 3, t % 3].transpose(3, 0, 2, 1)
         .reshape(128, CT * 256) for t in (PAIR_T, PAIR_T2)], axis=1),
        -240, 240).astype(NPF8))

    nc = _get_program()
    in_maps = []
    for c in range(NCORES):
        sl = slice(S * c, S * (c + 1))
        in_maps.append({
            "xpad": np.ascontiguousarray(xpad[sl]),
            "x8": np.ascontiguousarray(x8[sl]), "w1p8": w1p8,
            "w1sb": w1sb, "bank": bank, "wrt": wrt, "brb": brb,
            "b1sb": b1sb, "b2sb": b2sb,
        })

    res = run_bass_kernel_spmd(
        nc, in_maps, core_ids=list(range(NCORES)), trace=TRACE)
    LAST_EXEC_NS = res.exec_time_ns

    out = np.empty((B, C, H, W), f32)
    for c in range(NCORES):
        out[S * c:S * (c + 1)] = res.results[c]["out"].reshape(
            S, C, H, W).astype(f32)
    return out


if __name__ == "__main__":
    rng = np.random.default_rng(0)
    ins = {
        "x": rng.standard_normal((B, C, H, W), f32 := np.float32),
        "w1": rng.standard_normal((C, C, KH, KW), f32) * 0.05,
        "bn1_g": np.ones(C, f32), "bn1_b": np.zeros(C, f32),
        "bn1_m": rng.standard_normal(C, f32) * 0.05,
        "bn1_v": np.abs(rng.standard_normal(C, f32) * 0.05) + 1.0,
        "wr": rng.standard_normal((E, C), f32) * 0.05,
        "br": np.zeros(E, f32),
        "w_e": rng.standard_normal((E, C * C * KH * KW), f32) * 0.05,
        "bn2_g": np.ones(C, f32), "bn2_b": np.zeros(C, f32),
        "bn2_m": rng.standard_normal(C, f32) * 0.05,
        "bn2_v": np.abs(rng.standard_normal(C, f32) * 0.05) + 1.0,
    }
    o = kernel(**ins)
    print(o.shape, o.dtype)



# revision 5
# speedup vs baseline: 1.1604x; 1.1604x over previous
"""Trainium2 Bass kernel for Bottleneck+DynamicConv (B=16,C=256,H=W=64,E=4).

Data-parallel over batch: 8 NeuronCores x 2 samples each. Both 3x3 convs run
as 1D Winograd F(2,3) along H (direct in W): for each tile-row pair the four
B^T row-combinations T[u] are built on the vector engine (all +-1 coeffs,
fp16 2x-mode tensor_tensor ops), the PE contracts U[u,dx] @ T[u] (24 matmuls
of 512 free per strip-o instead of direct conv's 36), psum M[u] is evacuated
by the scalar engine as fp16, and the A^T combination (+-1) runs on the
vector engine. This cuts PE work by 1/3 vs direct fp16 convolution while
staying fp16 end to end (rel err ~1e-3; fp8 points measurably exceed the
2e-2 gate in winograd space, so none are used).

Per (sample, conv, o): 4 strips of 8 tile-rows; psum tile [128, 4u, 512]
(4 banks), two in flight. Conv1 weights are G-transformed on the host; for
conv2 the expert bank is mixed in direct space (stt with routing-gate AP
scalars) and u1 = 0.5(w0+w1+w2) / u2 = 0.5(w0-w1+w2) are built on-device;
u0/u3 alias the mixed ky0/ky2 blocks directly. Routing pools y through the
SiLU epilogue's accum_out, so no separate image reduction is needed. T
halves and kern prep are emitted ahead of the consuming strips so the PE
stream stays dense across the conv1(s0)->conv1(s1)->conv2(s0)->conv2(s1)
sequence.
"""

from contextlib import ExitStack

import numpy as np

import concourse.bacc as bacc
import concourse.bass as bass
import concourse.mybir as mybir
from concourse import tile
from concourse.bass_utils import run_bass_kernel_spmd

B, C, H, W, E = 16, 256, 64, 64, 4
KH = KW = 3
EPS = 1e-5
NCORES = 8
S = B // NCORES           # samples per core = 2
CT = C // 128             # channel tiles = 2
PD = W + 2                # padded width/height = 66
PF = PD * PD              # padded flat pixels per channel tile = 4356
HWF = H * W               # 4096
NU = 4                    # winograd points per tile-row pair
NDX = 3                   # direct column taps
NSTRIP = 4                # strips per (sample, conv, o); 8 tile-rows each
TPS = 8                   # tile-rows per strip
NN = TPS * W              # matmul free dim = 512
THALF = NU * 16 * PD      # T half tile cols = 4224
W1COLS = CT * NU * NDX * CT * 128   # 6144
KDCOLS = KH * CT * NDX * CT * 128   # 4608 direct blocks (ky, o, dx, ci)
KYB = KDCOLS // 3                   # 1536 = one ky block group
F16 = mybir.dt.float16
F32 = mybir.dt.float32
NPF16 = np.float16
Alu = mybir.AluOpType

TRACE = False
LAST_EXEC_NS = None
ACT_FUNC = mybir.ActivationFunctionType.Silu

_prog_cache = {}


def _build_program():
    nc = bacc.Bacc(
        "TRN2", target_bir_lowering=False, debug=False,
        enable_asserts=False, num_devices=NCORES)

    xpad_d = nc.dram_tensor("xpad", [S, CT, 128, PF], F16, kind="ExternalInput")
    w1u_d = nc.dram_tensor("w1u", [128, W1COLS], F16, kind="ExternalInput")
    bank_d = nc.dram_tensor("bank", [128, E * KDCOLS], F16, kind="ExternalInput")
    wr_d = nc.dram_tensor("wrt", [128, CT * E], F32, kind="ExternalInput")
    br_d = nc.dram_tensor("brb", [128, E], F32, kind="ExternalInput")
    b1_d = nc.dram_tensor("b1sb", [128, CT], F32, kind="ExternalInput")
    b2_d = nc.dram_tensor("b2sb", [128, CT], F32, kind="ExternalInput")
    out_d = nc.dram_tensor("out", [S, CT, 128, HWF], F16, kind="ExternalOutput")

    with tile.TileContext(nc) as tc, ExitStack() as ctx:
        const = ctx.enter_context(tc.tile_pool(name="const", bufs=1))
        xp_pool = ctx.enter_context(tc.tile_pool(name="xp", bufs=2))
        yp_pool = ctx.enter_context(tc.tile_pool(name="yp", bufs=2))
        t_pool = ctx.enter_context(tc.tile_pool(name="tp", bufs=2))
        kd_pool = ctx.enter_context(tc.tile_pool(name="kd", bufs=2))
        ku_pool = ctx.enter_context(tc.tile_pool(name="ku", bufs=4))
        m_pool = ctx.enter_context(tc.tile_pool(name="m16", bufs=2))
        a_pool = ctx.enter_context(tc.tile_pool(name="ast", bufs=2))
        o_pool = ctx.enter_context(tc.tile_pool(name="ost", bufs=2))
        small = ctx.enter_context(tc.tile_pool(name="small", bufs=2))
        ps_pool = ctx.enter_context(tc.tile_pool(name="ps", bufs=2, space="PSUM"))

        # HAM warmup: burn the NEFF-preamble DMA window on dummy matmuls so
        # the PE clock-gate is fully open when real work starts.
        dummy_t = const.tile([128, 128], F16)
        nc.gpsimd.memset(dummy_t[:], 0.0)
        warm_ps = ps_pool.tile([128, NU, NN], F32, tag="M")
        for _ in range(36):
            nc.tensor.matmul(
                warm_ps[:, 0:1, 0:128], dummy_t[:], dummy_t[:],
                start=True, stop=True)

        # constants + conv1 winograd weights (o=0 half first: it gates the
        # first psum group)
        w1u_t = const.tile([128, W1COLS], F16)
        HC = W1COLS // 2
        nc.sync.dma_start(w1u_t[:, 0:HC], w1u_d.ap()[:, 0:HC])
        b1_t = const.tile([128, CT], F32)
        nc.sync.dma_start(b1_t[:], b1_d.ap())
        wr_t = const.tile([128, CT * E], F32)
        br_t = const.tile([128, E], F32)
        b2_t = const.tile([128, CT], F32)
        ones_t = const.tile([128, 128], F32)
        nc.vector.memset(ones_t[:], 1.0)
        half_t = const.tile([128, 1], F32)
        nc.vector.memset(half_t[:], 0.5)
        bank_t = const.tile([128, E * KDCOLS], F16)

        # input DMA in consumption order: s0 rows 0..33 both ci (gates the
        # first T ops), w1u second half, s0 rows 34..65, then s1, then the
        # expert bank (needed only after conv1(s0)'s routing), split rings.
        RH = 34 * PD
        xpts = [xp_pool.tile([128, CT * PF], F16, tag="xp", name=f"xp{i}")
                for i in range(S)]
        for ci, eng in ((0, nc.sync), (1, nc.scalar)):
            off = ci * PF
            eng.dma_start(xpts[0][:, off:off + RH],
                          xpad_d.ap()[0, ci][:, 0:RH])
        nc.sync.dma_start(w1u_t[:, HC:], w1u_d.ap()[:, HC:])
        for ci, eng in ((0, nc.sync), (1, nc.scalar)):
            off = ci * PF
            eng.dma_start(xpts[0][:, off + RH:off + PF],
                          xpad_d.ap()[0, ci][:, RH:PF])
        nc.sync.dma_start(wr_t[:], wr_d.ap())
        nc.sync.dma_start(br_t[:], br_d.ap())
        nc.sync.dma_start(b2_t[:], b2_d.ap())
        for ci, eng in ((0, nc.sync), (1, nc.scalar)):
            eng.dma_start(xpts[1][:, ci * PF:(ci + 1) * PF],
                          xpad_d.ap()[1, ci])
        EB = E * KDCOLS // 2
        nc.sync.dma_start(bank_t[:, 0:EB], bank_d.ap()[:, 0:EB])
        nc.scalar.dma_start(bank_t[:, EB:], bank_d.ap()[:, EB:])

        # y tiles + pad-ring zeroing upfront (disjoint from the interior the
        # SiLU epilogues write, so no false ordering)
        yts, paccs, kcols = [], [], []
        for s in range(S):
            yt = yp_pool.tile([128, CT * PF], F16, tag="yp")
            yts.append(yt)
            for ci in range(CT):
                yv = yt[:, ci * PF:(ci + 1) * PF].rearrange(
                    "p (h w) -> p h w", h=PD)
                nc.gpsimd.memset(yv[:, 0:1, :], 0.0)
                nc.gpsimd.memset(yv[:, PD - 1:PD, :], 0.0)
                nc.gpsimd.memset(yv[:, :, 0:1], 0.0)
                nc.gpsimd.memset(yv[:, :, PD - 1:PD], 0.0)
            pacc = small.tile([128, CT * NSTRIP], F32, tag=f"pacc{s}", bufs=1)
            paccs.append(pacc)

        def w1col(u, dx, o, ci):
            blk = ((o * NU + u) * NDX + dx) * CT + ci
            return w1u_t[:, blk * 128:(blk + 1) * 128]

        def make_kcol(kd_t, u1_t, u2_t):
            def kcol(u, dx, o, ci):
                if u == 0 or u == 3:
                    ky = 0 if u == 0 else 2
                    blk = ((ky * CT + o) * NDX + dx) * CT + ci
                    return kd_t[:, blk * 128:(blk + 1) * 128]
                t = u1_t if u == 1 else u2_t
                blk = (o * NDX + dx) * CT + ci
                return t[:, blk * 128:(blk + 1) * 128]
            return kcol

        def emit_T(src_tile, ci, h0, dst):
            """B^T row transform for 16 tile-rows starting at tile-row h0:
            dst[u] [128, 16, 66] from padded rows 2*(h0+t)+k."""
            xr = src_tile[:, ci * PF:(ci + 1) * PF].rearrange(
                "p (t f w) -> p t f w", t=33, f=2)
            dv = [dst[:, u * 16 * PD:(u + 1) * 16 * PD].rearrange(
                "p (t f w) -> p t f w", t=16, f=1) for u in range(NU)]
            d0 = xr[:, h0:h0 + 16, 0:1, :]
            d1 = xr[:, h0:h0 + 16, 1:2, :]
            d2 = xr[:, h0 + 1:h0 + 17, 0:1, :]
            d3 = xr[:, h0 + 1:h0 + 17, 1:2, :]
            nc.vector.tensor_sub(dv[0], d0, d2)
            nc.vector.tensor_add(dv[1], d1, d2)
            nc.vector.tensor_sub(dv[2], d2, d1)
            nc.vector.tensor_sub(dv[3], d1, d3)

        def emit_T_half(src_tile, half):
            tiles = {}
            for ci in range(CT):
                dst = t_pool.tile([128, THALF], F16, tag=f"T{ci}")
                emit_T(src_tile, ci, half * 16, dst)
                tiles[ci] = dst
            return tiles

        def emit_routing_mix(s):
            """pooled -> routing gates -> mixed direct kern -> u1/u2."""
            pacc = paccs[s]
            r_t = small.tile([128, E], F32, tag="r")
            psr = ps_pool.tile([128, NU, NN], F32, tag="M")
            for o in range(CT):
                pooled = small.tile([128, 1], F32, tag="pooled")
                nc.vector.tensor_reduce(
                    pooled[:], pacc[:, o * NSTRIP:(o + 1) * NSTRIP],
                    axis=mybir.AxisListType.X, op=Alu.add)
                pbc = small.tile([128, 128], F32, tag="pbc")
                nc.vector.tensor_scalar_mul(pbc[:], ones_t[:], pooled[:, 0:1])
                nc.tensor.matmul(
                    psr[:, 0:1, 0:E], pbc[:], wr_t[:, o * E:(o + 1) * E],
                    start=(o == 0), stop=(o == CT - 1))
            logits = small.tile([128, E], F32, tag="logits")
            nc.vector.tensor_add(
                logits[:].rearrange("p (f e) -> p f e", f=1),
                psr[:, 0:1, 0:E],
                br_t[:].rearrange("p (f e) -> p f e", f=1))
            nc.scalar.activation(
                r_t[:], logits[:], mybir.ActivationFunctionType.Sigmoid)

            kd_t = kd_pool.tile([128, KDCOLS], F16, tag="kd")
            nc.vector.tensor_scalar_mul(
                kd_t[:], bank_t[:, 0:KDCOLS], r_t[:, 0:1])
            for e in range(1, E):
                nc.vector.scalar_tensor_tensor(
                    kd_t[:], bank_t[:, e * KDCOLS:(e + 1) * KDCOLS],
                    r_t[:, e:e + 1], kd_t[:], Alu.mult, Alu.add)
            # u1 = 0.5(ky0+ky1+ky2), u2 = 0.5(ky0-ky1+ky2)
            p_t = small.tile([128, KYB], F16, tag="ktmp_p", bufs=1)
            h_t = small.tile([128, KYB], F16, tag="ktmp_h", bufs=1)
            nc.vector.tensor_add(p_t[:], kd_t[:, 0:KYB], kd_t[:, 2 * KYB:])
            nc.vector.tensor_scalar_mul(
                h_t[:], kd_t[:, KYB:2 * KYB], half_t[:, 0:1])
            u1_t = ku_pool.tile([128, KYB], F16, tag="ku")
            u2_t = ku_pool.tile([128, KYB], F16, tag="ku")
            nc.vector.scalar_tensor_tensor(
                u1_t[:], p_t[:], half_t[:, 0:1], h_t[:], Alu.mult, Alu.add)
            nc.vector.scalar_tensor_tensor(
                u2_t[:], p_t[:], half_t[:, 0:1], h_t[:], Alu.mult,
                Alu.subtract)
            kcols.append(make_kcol(kd_t, u1_t, u2_t))

        def emit_conv(s, conv, hooks, pre_h0=None):
            """One conv layer for sample s. conv=0: x->y (silu+pool accum);
            conv=1: y->out (silu+residual+DMA). hooks[i] emitted after strip
            i's two psum groups (routing/mix and T prefetch for later
            convs). pre_h0: T-half-0 tiles already emitted by an earlier
            hook."""
            src = xpts[s] if conv == 0 else yts[s]
            bias_t = b1_t if conv == 0 else b2_t
            t_half = {0: pre_h0 if pre_h0 is not None
                      else emit_T_half(src, 0)}

            for strip in range(NSTRIP):
                if strip == 2:
                    t_half[1] = emit_T_half(src, 1)
                half = strip // 2
                t0 = (strip % 2) * TPS
                wcol = w1col if conv == 0 else kcols[s]
                for o in range(CT):
                    ps = ps_pool.tile([128, NU, NN], F32, tag="M")
                    for u in range(NU):
                        tvs = [t_half[half][ci][
                            :, u * 16 * PD:(u + 1) * 16 * PD].rearrange(
                            "p (t w) -> p t w", t=16) for ci in range(CT)]
                        idx = 0
                        for dx in range(NDX):
                            for ci in range(CT):
                                nc.tensor.matmul(
                                    ps[:, u:u + 1, :], wcol(u, dx, o, ci),
                                    tvs[ci][:, t0:t0 + TPS, dx:dx + W],
                                    start=(idx == 0), stop=(idx == 5))
                                idx += 1
                    m16 = m_pool.tile([128, NU * NN], F16, tag="m16")
                    nc.scalar.copy(
                        m16[:].rearrange("p (u n) -> p u n", u=NU), ps[:])
                    mv = [m16[:, u * NN:(u + 1) * NN].rearrange(
                        "p (t f w) -> p t f w", t=TPS, f=1)
                        for u in range(NU)]
                    ast = a_pool.tile([128, TPS * 2 * W], F16, tag="ast")
                    av = ast.rearrange("p (t a w) -> p t a w", t=TPS, a=2)
                    a0 = av[:, :, 0:1, :]
                    a1 = av[:, :, 1:2, :]
                    nc.vector.tensor_add(a0, mv[0], mv[1])
                    nc.vector.tensor_add(a0, a0, mv[2])
                    nc.vector.tensor_sub(a1, mv[1], mv[2])
                    nc.vector.tensor_sub(a1, a1, mv[3])
                    r0 = strip * 2 * TPS
                    if conv == 0:
                        yr = yts[s][:, o * PF + PD:o * PF + PD + 64 * PD]\
                            .rearrange("p (t f w) -> p t f w", t=32, f=2)
                        nc.scalar.activation(
                            yr[:, strip * TPS:(strip + 1) * TPS, :, 1:1 + W],
                            av, ACT_FUNC, bias=bias_t[:, o:o + 1],
                            accum_out=paccs[s][:, o * NSTRIP + strip:
                                               o * NSTRIP + strip + 1])
                    else:
                        ost = o_pool.tile([128, 2 * TPS * W], F16, tag="ost")
                        ov = ost.rearrange("p (t f w) -> p t f w", t=TPS, f=2)
                        nc.scalar.activation(
                            ov, av, ACT_FUNC, bias=bias_t[:, o:o + 1])
                        xr = xpts[s][:, o * PF + (1 + r0) * PD:
                                     o * PF + (1 + r0 + 16) * PD].rearrange(
                            "p (t w) -> p t w", t=16)
                        orow = ost.rearrange("p (t w) -> p t w", t=16)
                        nc.vector.tensor_add(orow, orow, xr[:, :, 1:1 + W])
                        eng = [nc.gpsimd, nc.sync][(strip + o) % 2]
                        eng.dma_start(
                            out_d.ap()[s, o][:, r0 * W:(r0 + 16) * W],
                            ost[:])
                hook = hooks.get(strip)
                if hook is not None:
                    hook()

        nxt = {}
        emit_conv(0, 0, {2: lambda: nxt.__setitem__(
            "c1s1", emit_T_half(xpts[1], 0))})
        emit_conv(1, 0, {0: lambda: emit_routing_mix(0),
                         2: lambda: nxt.__setitem__(
                             "c2s0", emit_T_half(yts[0], 0))},
                  pre_h0=nxt["c1s1"])
        emit_conv(0, 1, {0: lambda: emit_routing_mix(1),
                         2: lambda: nxt.__setitem__(
                             "c2s1", emit_T_half(yts[1], 0))},
                  pre_h0=nxt["c2s0"])
        emit_conv(1, 1, {}, pre_h0=nxt["c2s1"])

    nc.compile()
    return nc


def _get_program():
    if "nc" not in _prog_cache:
        _prog_cache["nc"] = _build_program()
    return _prog_cache["nc"]


# F(2,3) weight transform G (winograd rows u from conv taps ky)
_G = np.array([[1, 0, 0], [.5, .5, .5], [.5, -.5, .5], [0, 0, 1]], np.float32)


def kernel(x, w1, bn1_g, bn1_b, bn1_m, bn1_v, wr, br, w_e,
           bn2_g, bn2_b, bn2_m, bn2_v):
    global LAST_EXEC_NS
    f32 = np.float32
    x = np.ascontiguousarray(np.asarray(x, f32))
    w1 = np.asarray(w1, f32)
    wr = np.asarray(wr, f32)
    br = np.asarray(br, f32)
    w_e = np.asarray(w_e, f32)

    s1 = np.asarray(bn1_g, f32) / np.sqrt(np.asarray(bn1_v, f32) + EPS)
    b1 = np.asarray(bn1_b, f32) - np.asarray(bn1_m, f32) * s1
    s2 = np.asarray(bn2_g, f32) / np.sqrt(np.asarray(bn2_v, f32) + EPS)
    b2 = np.asarray(bn2_b, f32) - np.asarray(bn2_m, f32) * s2

    # conv1 weights: BN1 scale fold, winograd G-transform over ky, lhsT
    # layout [cin_p, (o, u, dx, ci, cout)]
    w1f = (w1 * s1[:, None, None, None]).reshape(CT, 128, CT, 128, KH, KW)
    w1uf = np.einsum('uk,apbqkd->qaudbp', _G, w1f)
    w1u = np.ascontiguousarray(w1uf.reshape(128, W1COLS)).astype(NPF16)

    # expert bank in direct space, ky-major: [cin_p, e, (ky, o, dx, ci, cout)]
    wef = (w_e.reshape(E, C, C, KH, KW)
           * s2[None, :, None, None, None]).reshape(E, CT, 128, CT, 128,
                                                    KH, KW)
    bank = np.ascontiguousarray(
        wef.transpose(4, 0, 5, 1, 6, 3, 2).reshape(128, E * KDCOLS)
    ).astype(NPF16)

    wrt = np.ascontiguousarray(
        (wr / HWF).reshape(E, CT, 128).transpose(2, 1, 0).reshape(128, CT * E))
    brb = np.ascontiguousarray(np.broadcast_to(br, (128, E)))
    b1sb = np.ascontiguousarray(b1.reshape(CT, 128).T)
    b2sb = np.ascontiguousarray(b2.reshape(CT, 128).T)

    pad = np.zeros((B, CT, 128, PD, PD), f32)
    pad[:, :, :, 1:H + 1, 1:W + 1] = x.reshape(B, CT, 128, H, W)
    xpad = np.ascontiguousarray(pad.reshape(B, CT, 128, PF).astype(NPF16))

    nc = _get_program()
    in_maps = []
    for c in range(NCORES):
        sl = slice(S * c, S * (c + 1))
        in_maps.append({
            "xpad": np.ascontiguousarray(xpad[sl]),
            "w1u": w1u, "bank": bank, "wrt": wrt, "brb": brb,
            "b1sb": b1sb, "b2sb": b2sb,
        })

    res = run_bass_kernel_spmd(
        nc, in_maps, core_ids=list(range(NCORES)), trace=TRACE)
    LAST_EXEC_NS = res.exec_time_ns

    out = np.empty((B, C, H, W), f32)
    for c in range(NCORES):
        out[S * c:S * (c + 1)] = res.results[c]["out"].reshape(
            S, C, H, W).astype(f32)
    return out


if __name__ == "__main__":
    rng = np.random.default_rng(0)
    f32 = np.float32
    ins = {
        "x": rng.standard_normal((B, C, H, W), f32),
        "w1": rng.standard_normal((C, C, KH, KW), f32) * 0.05,
        "bn1_g": np.ones(C, f32), "bn1_b": np.zeros(C, f32),
        "bn1_m": rng.standard_normal(C, f32) * 0.05,
        "bn1_v": np.abs(rng.standard_normal(C, f32) * 0.05) + 1.0,
        "wr": rng.standard_normal((E, C), f32) * 0.05,
        "br": np.zeros(E, f32),
        "w_e": rng.standard_normal((E, C * C * KH * KW), f32) * 0.05,
        "bn2_g": np.ones(C, f32), "bn2_b": np.zeros(C, f32),
        "bn2_m": rng.standard_normal(C, f32) * 0.05,
        "bn2_v": np.abs(rng.standard_normal(C, f32) * 0.05) + 1.0,
    }
    o = kernel(**ins)
    print(o.shape, o.dtype)
